# revision 1
# baseline (speedup 1.0000x reference)
"""Trainium2 Bass kernel for batched dot-product attention with query-row
masking (nn_DotProductAttention).

Problem (hardcoded): B=16, N=2048, D=128, fp32.
  scores = Q @ K^T / sqrt(D)                  [B, N, N]
  scores[b, q, :] = -1e6  where q >= valid_lens[b]   (masks whole query ROWS)
  attn = softmax(scores, axis=-1)
  out = attn @ V                              [B, N, D]

A fully-masked row softmaxes to the uniform distribution (softmax of a
constant row), so zeroing the Q rows (making the score row constant 0)
produces the identical result: exp(0)=1 -> 1/N weights. We therefore
multiply Q rows by a 0/1 mask and run plain softmax without max
subtraction (scores are ~N(0,1): exp never overflows).

Sharding: batch dim across 8 cores, 2 batches per core. Per batch, per
128-key chunk c: S^T_c[k,q] = (K^T_c)^T-stationary matmul over Q^T
(fp32r, 1 cyc/row), exp on ACT (PSUM->SBUF), then accumulate
O^T += V_c^T @ E^T_c and replicated row-sums += ones^T @ E^T_c in PSUM.
Normalize O^T with reciprocal sums (both [d/128-replicated, q] layout),
PE-transpose back to [q, d] and DMA out.
"""

import os

os.environ.setdefault("JAX_PLATFORMS", "")

import math

import numpy as np

import concourse.bass as bass
import concourse.mybir as mybir
import concourse.tile as tile
from concourse import bacc
from concourse.bass import ts
from concourse.bass_utils import run_bass_kernel_spmd
from concourse.masks import make_identity

N_CORES = 8
B = 16
N = 2048
D = 128
NB = B // N_CORES  # batches per core
KC = N // 128  # key chunks of 128
QBLK = 1024  # q block size
NQB = N // QBLK
SCALE = 1.0 / math.sqrt(D)

F32 = mybir.dt.float32
F32R = mybir.dt.float32r

# module-level knob: number of times the per-core compute is replicated
# inside the program (used by test.py for slope-based HW timing)
_REPLICATE = 1

_nc_cache = {}


def r32(ap):
    return ap.bitcast(F32R)


def build_program(replicate=1, *, mm_dt=None, e_dt=None, do_s=True, do_exp=True,
                  do_o=True, do_sums=True, do_fin=True, sums_mode="replicated",
                  prep_first=True, et_bufs=6):
    if mm_dt is None:
        mm_dt = F32R
    if e_dt is None:
        e_dt = mybir.dt.bfloat16
    nc = bacc.Bacc("TRN2", target_bir_lowering=False, debug=False, num_devices=N_CORES)

    q_d = nc.dram_tensor("q", [NB, N, D], F32, kind="ExternalInput")
    k_d = nc.dram_tensor("k", [NB, N, D], F32, kind="ExternalInput")
    v_d = nc.dram_tensor("v", [NB, N, D], F32, kind="ExternalInput")
    m_d = nc.dram_tensor("mask", [NB, N], F32, kind="ExternalInput")
    o_d = nc.dram_tensor("out", [NB, N, D], F32, kind="ExternalOutput")

    with tile.TileContext(nc) as tc:
        with (
            tc.tile_pool(name="consts", bufs=1) as consts,
            tc.tile_pool(name="nat", bufs=2) as nat,  # natural-layout staging
            tc.tile_pool(name="tposed", bufs=2) as tp,  # Q^T/K^T/V tiles
            tc.tile_pool(name="stage", bufs=4) as stage,
            tc.tile_pool(name="et", bufs=et_bufs) as etp,
            tc.tile_pool(name="fin", bufs=2) as fin,
            tc.tile_pool(name="psS", bufs=2, space="PSUM") as psS,
            tc.tile_pool(name="psO", bufs=1, space="PSUM") as psO,
            tc.tile_pool(name="psSum", bufs=1, space="PSUM") as psSum,
        ):
            ident = consts.tile([128, 128], F32)
            make_identity(nc, ident[:])
            ones_f = consts.tile([128, 128], F32)
            nc.vector.memset(ones_f[:], 1.0)
            ones_w = 32 if sums_mode == "colpack" else 128
            ones = consts.tile([128, ones_w], e_dt)
            nc.vector.tensor_copy(ones[:], ones_f[:, :ones_w])

            def emit_prep(b, store):
                if True:
                    # ---- load natural-layout inputs ----
                    qnat = nat.tile([128, KC, D], F32, tag="qnat")
                    nc.sync.dma_start(
                        qnat[:], q_d[b].rearrange("(c p) d -> p c d", p=128)
                    )
                    knat = nat.tile([128, KC, D], F32, tag="knat")
                    nc.sync.dma_start(
                        knat[:], k_d[b].rearrange("(c p) d -> p c d", p=128)
                    )
                    vs = nat.tile([128, KC, D], F32, tag="vs")
                    nc.sync.dma_start(vs[:], v_d[b].rearrange("(c p) d -> p c d", p=128))
                    vsr = tp.tile([128, KC, D], e_dt, tag="vsr")
                    nc.vector.tensor_copy(vsr[:], vs[:])
                    mk = nat.tile([128, KC], F32, tag="mk")
                    nc.sync.dma_start(mk[:], m_d[b].rearrange("(c p) -> p c", p=128))

                    # ---- build Q^T (masked) and K^T via PE transposes ----
                    qt = tp.tile([128, N], mm_dt, tag="qt")  # [d, q]
                    kt = tp.tile([128, N], mm_dt, tag="kt")  # [d, k]
                    for g in range(2):  # two groups of 8 tiles -> one psum tile
                        pq = psS.tile([128, QBLK], F32, tag="st")
                        pk = psS.tile([128, QBLK], F32, tag="st")
                        for j in range(8):
                            i = 8 * g + j
                            qm = stage.tile([128, D], F32, tag="qm")
                            nc.vector.tensor_scalar_mul(
                                qm[:], qnat[:, i, :], mk[:, i : i + 1]
                            )
                            nc.tensor.transpose(pq[:, ts(j, 128)], qm[:], ident[:])
                            nc.tensor.transpose(
                                pk[:, ts(j, 128)], knat[:, i, :], ident[:]
                            )
                        nc.vector.tensor_copy(qt[:, ts(g, QBLK)], pq[:])
                        nc.vector.tensor_copy(kt[:, ts(g, QBLK)], pk[:])

                    store[b] = (qt, kt, vsr)

            def emit_main(b, store):
                qt, kt, vsr = store[b]
                if True:
                    # ---- main attention loops ----
                    if do_sums and sums_mode == "colpack":
                        # one bank: band j=2*qb+h holds sums for that q-half
                        smb = psSum.tile([128, 512], F32, tag="sums")
                    for qb in range(NQB):
                        ot = psO.tile([128, QBLK], F32, tag="ot")  # O^T accum
                        if do_sums and sums_mode == "replicated":
                            sm = psSum.tile([128, QBLK], F32, tag="sums")
                        for c in range(KC):
                            st = psS.tile([128, QBLK], F32, tag="st")
                            if do_s:
                                for h in range(2):
                                    nc.tensor.matmul(
                                        st[:, ts(h, 512)],
                                        kt[:, ts(c, 128)],
                                        qt[:, ts(2 * qb + h, 512)],
                                        start=True,
                                        stop=True,
                                    )
                            et = etp.tile([128, QBLK], e_dt, tag="et")
                            if do_exp:
                                nc.scalar.activation(
                                    et[:],
                                    st[:],
                                    mybir.ActivationFunctionType.Exp,
                                    scale=SCALE,
                                )
                            elif do_o or do_sums:
                                nc.vector.memset(et[:], 0.001)
                            first, last = c == 0, c == KC - 1
                            for h in range(2):
                                if do_o:
                                    nc.tensor.matmul(
                                        ot[:, ts(h, 512)],
                                        vsr[:, c, :],
                                        et[:, ts(h, 512)],
                                        start=first,
                                        stop=last,
                                    )
                                if do_sums and sums_mode == "replicated":
                                    nc.tensor.matmul(
                                        sm[:, ts(h, 512)],
                                        ones[:],
                                        et[:, ts(h, 512)],
                                        start=first,
                                        stop=last,
                                    )
                                elif do_sums:
                                    j = 2 * qb + h
                                    nc.tensor.matmul(
                                        smb[32 * j : 32 * (j + 1), :],
                                        ones[:],
                                        et[:, ts(h, 512)],
                                        start=first,
                                        stop=last,
                                        tile_position=(0, 32 * j),
                                        skip_group_check=True,
                                    )
                        # normalize: O^T * (1/sums), both [128-replicated, q]
                        if not do_fin:
                            continue
                        rs = stage.tile([128, QBLK], F32, tag="rs")
                        if do_sums and sums_mode == "colpack":
                            for h in range(2):
                                j = 2 * qb + h
                                r32t = stage.tile([32, 512], F32, tag="r32t")
                                nc.vector.reciprocal(
                                    r32t[:], smb[32 * j : 32 * (j + 1), :]
                                )
                                nc.gpsimd.partition_broadcast(
                                    rs[:, ts(h, 512)], r32t[0:1, :]
                                )
                        elif do_sums:
                            nc.vector.reciprocal(rs[:], sm[:])
                        else:
                            nc.vector.memset(rs[:], 1.0)
                        ont = stage.tile([128, QBLK], F32, tag="ont")
                        if do_o:
                            nc.vector.tensor_mul(ont[:], ot[:], rs[:])
                        else:
                            nc.vector.memset(ont[:], 0.5)
                        # transpose back to [q, d]
                        pto = psS.tile([128, QBLK], F32, tag="st")
                        for j in range(8):
                            nc.tensor.transpose(
                                pto[:, ts(j, 128)], ont[:, ts(j, 128)], ident[:]
                            )
                        ob = fin.tile([128, 8, 128], F32, tag="ob")
                        nc.vector.tensor_copy(
                            ob[:], pto[:].rearrange("p (j d) -> p j d", j=8)
                        )
                        nc.sync.dma_start(
                            o_d[b, ts(qb, QBLK), :].rearrange(
                                "(j p) d -> p j d", p=128
                            ),
                            ob[:],
                        )

            def emit_body():
                store = {}
                if prep_first:
                    for b in range(NB):
                        emit_prep(b, store)
                    for b in range(NB):
                        emit_main(b, store)
                else:
                    for b in range(NB):
                        emit_prep(b, store)
                        emit_main(b, store)

            if replicate == 1:
                emit_body()
            else:
                with tc.For_i(0, replicate, 1):
                    emit_body()

    nc.compile()
    return nc


def _get_nc(replicate):
    if replicate not in _nc_cache:
        _nc_cache[replicate] = build_program(replicate)
    return _nc_cache[replicate]


def _make_in_maps(queries, keys, values, valid_lens):
    queries = np.asarray(queries, dtype=np.float32)
    keys = np.asarray(keys, dtype=np.float32)
    values = np.asarray(values, dtype=np.float32)
    valid_lens = np.asarray(valid_lens, dtype=np.int32)
    mask = (np.arange(N)[None, :] < valid_lens[:, None]).astype(np.float32)
    in_maps = []
    for i in range(N_CORES):
        sl = slice(NB * i, NB * (i + 1))
        in_maps.append(
            {
                "q": np.ascontiguousarray(queries[sl]),
                "k": np.ascontiguousarray(keys[sl]),
                "v": np.ascontiguousarray(values[sl]),
                "mask": np.ascontiguousarray(mask[sl]),
            }
        )
    return in_maps


def kernel(queries, keys, values, valid_lens):
    nc = _get_nc(_REPLICATE)
    in_maps = _make_in_maps(queries, keys, values, valid_lens)
    res = run_bass_kernel_spmd(nc, in_maps, core_ids=list(range(N_CORES)))
    out = np.concatenate(
        [res.results[i]["out"] for i in range(N_CORES)], axis=0
    ).astype(np.float32)
    return out

